# revision 9
# baseline (speedup 1.0000x reference)
"""YOLO-style detection decode (nms_detection) on 8 trn2 NeuronCores.

Data-parallel over batch (64 -> 8 images/core). The host packs each
core's inputs into ONE flat f32 DRAM tensor, pre-transposed per scale
to cell-major chunks: [128 partitions, nch * 255] where partition p,
column 255*j + c holds channel c of cell j*128 + p. Every device load
is then one large contiguous-stride DMA, and no on-device transpose is
needed at all (the memory-regime roofline is the 29 MB input stream).
The result is ONE [128, 4014] f32 tensor (chunk-major cells on
partitions, 18 = 3 anchors x 6 box floats per cell), reassembled on
the host.

Device pipeline per 32-chunk group (chunk = 128 cells), all reads
straight from the input strip in SBUF:
  - scalar engine copies the 5 box channels (conf,x,y,w,h per anchor)
    into a per-scale accumulator.
  - DVE reduce_max over the 80 class cols per anchor -> m (exact).
  - DVE computes cls - m into an SBUF scratch (exact at the top:
    x - x = 0, Sterbenz near the max), then adds (79-c)*2^-31. The
    winner's value is exactly (79-argmax)*2^-31 >= 0 while every loser
    stays < 0, so a second DVE reduce_max recovers argmax exactly
    (incl. first-index ties, matching jnp.argmax).
  - Decode (sigmoid, grid offsets, exp*anchor, conf mask) runs ONCE
    per scale as ~10 wide DVE/ACT ops over the accumulators, writing
    the output accumulator in final layout; one DMA stores it.
"""

import os
from contextlib import ExitStack

import numpy as np

import concourse.bass as bass
import concourse.tile as tile
from concourse import bacc, mybir
from concourse.bass_utils import run_bass_kernel_spmd

N_CORES = 8
B = 64
B_PER = B // N_CORES
CASE = 416.0
SCALES = [("52", 52, 8.0), ("26", 26, 16.0), ("13", 13, 32.0)]
CHUNK = 128
LDC = 32           # chunks per SBUF load strip / compute group
F32 = mybir.dt.float32
AX = mybir.AxisListType
OP = mybir.AluOpType
AF = mybir.ActivationFunctionType
IOTA_SCALE = 2.0 ** -31


def _cells(h):
    return B_PER * h * h


def _nchunks(h):
    return (_cells(h) + CHUNK - 1) // CHUNK


NCH = {tag: _nchunks(h) for tag, h, _ in SCALES}
NCH_TOT = sum(NCH.values())          # 223
OUT_W = NCH_TOT * 18                 # 4014 f32 per partition


def _gxy_section(h, t):
    """[128, 2*nch] grid offsets: cols (2j, 2j+1) = (gx, gy) of chunk j."""
    n = _cells(h)
    nch = _nchunks(h)
    cells = np.arange(nch * CHUNK)
    s = cells % (h * h)
    gx = (s % h).astype(np.float64) * t / CASE
    gy = (s // h).astype(np.float64) * t / CASE
    gx[cells >= n] = 0.0
    gy[cells >= n] = 0.0
    out = np.zeros((CHUNK, 2 * nch), np.float32)
    for j in range(nch):
        out[:, 2 * j] = gx[j * CHUNK:(j + 1) * CHUNK]
        out[:, 2 * j + 1] = gy[j * CHUNK:(j + 1) * CHUNK]
    return out


def _consts():
    iota = np.broadcast_to(
        ((79.0 - np.arange(80)) * IOTA_SCALE).astype(np.float32), (128, 80))
    gxy = np.concatenate([_gxy_section(h, t) for _, h, t in SCALES], axis=1)
    return {"gxy": gxy.astype(np.float32),
            "iota": np.ascontiguousarray(iota)}


_CONSTS = _consts()
_CONST_SHAPES = {"gxy": [128, 2 * NCH_TOT], "iota": [128, 80],
                 "anch": [128, 18]}

# packed input layout (f32 elements, per core): per-scale cell-major
# chunked activations, then the small constants.
_X_OFF = {}
_off = 0
for _tag, _h, _t in SCALES:
    _X_OFF[_tag] = _off
    _off += NCH[_tag] * 255 * CHUNK
_CONST_OFF = {}
for _name in ("gxy", "iota", "anch"):
    _CONST_OFF[_name] = _off
    _off += int(np.prod(_CONST_SHAPES[_name]))
TOTAL_IN = _off

# chunk-column base per scale in the accumulators / output
_J_OFF = {}
_off = 0
for _tag, _h, _t in SCALES:
    _J_OFF[_tag] = _off
    _off += NCH[_tag]


def _emit_scale(nc, tc, sb, acc, xin, h, t, tag):
    ST = int(os.environ.get("KSTAGE", "6"))
    TT = getattr(nc, os.environ.get("KTTENG", "vector"))
    nch = NCH[tag]
    J0 = _J_OFF[tag]
    k = float(t / CASE)
    p_in, p_cls, p_m, p_dec = acc["pools"]
    boxacc, idxacc, outacc = acc["boxacc"], acc["idxacc"], acc["outacc"]

    # [128, nch, 255]: partition p, chunk j, channel c = cell j*128+p
    xc = xin[_X_OFF[tag]:_X_OFF[tag] + nch * 255 * CHUNK] \
        .rearrange("(p j c) -> p j c", p=128, c=255)

    for jb in range(0, nch, LDC):
        lc = min(LDC, nch - jb)

        in_a = p_in.tile([128, LDC * 255], F32, tag="in_a")
        ia = in_a[:].rearrange("p (j c) -> p j c", c=255)[:, 0:lc, :]
        nc.sync.dma_start(ia, xc[:, jb:jb + lc, :])
        if ST < 1:
            continue

        iv = ia.rearrange("p j (a r) -> p j a r", a=3)   # [128, lc, 3, 85]
        cls_in = iv[:, :, :, 5:85]
        J = J0 + jb

        # box channels (conf,x,y,w,h per anchor) -> accumulator
        m_sb = p_m.tile([128, LDC * 3], F32, tag="m_sb")
        m_v = m_sb[:].rearrange("p (j a) -> p j a", j=LDC)[:, 0:lc, :]
        if ST >= 2:
            nc.scalar.copy(boxacc[:, J:J + lc], iv[:, :, :, 0:5])
            # ---- scan 1: exact per-anchor class max ----
            nc.vector.tensor_reduce(m_v, cls_in, axis=AX.X, op=OP.max)
        else:
            nc.vector.memset(m_sb[:, :], 0.0)

        # ---- recenter into scratch: cls - m, then + iota payload ----
        cls_s = p_cls.tile([128, LDC * 240], F32, tag="cls_s")
        cv = cls_s[:].rearrange("p (j a r) -> p j a r", j=LDC, a=3)[:, 0:lc]
        if ST >= 3:
            m_b = m_v.unsqueeze(3).broadcast_to([128, lc, 3, 80])
            TT.scalar_tensor_tensor(cv, cls_in, 1.0, m_b,
                                    op0=OP.mult, op1=OP.subtract)
        else:
            nc.vector.memset(cls_s[:, :], 0.0)
        if ST >= 4:
            i_b = sb["iota"].unsqueeze(1).unsqueeze(1) \
                .broadcast_to([128, lc, 3, 80])
            TT.tensor_tensor(cv, cv, i_b, op=OP.add)

        # ---- scan 2: argmax payload ----
        if ST >= 5:
            nc.vector.tensor_reduce(idxacc[:, J:J + lc], cv,
                                    axis=AX.X, op=OP.max)

    # ---- batched decode over the whole scale ----
    if ST < 6:
        return
    oX = acc["oX"]
    oA = outacc[:].rearrange("p (c a s) -> p c a s", a=3, s=6)[:, J0:J0 + nch]
    oT = outacc[:].rearrange("p (c a s) -> p c s a", a=3, s=6)[:, J0:J0 + nch]
    bx = boxacc[:, J0:J0 + nch]                   # [128, nch, 3, 5]

    econf = p_dec.tile([128, nch * 3], F32, tag=f"econf{tag}")
    e_v = econf[:].rearrange("p (c a) -> p c a", c=nch)
    nc.scalar.activation(e_v, bx[:, :, :, 0], AF.Exp, scale=-1.0)
    nc.vector.tensor_scalar(e_v, e_v, 1.0, None, op0=OP.add)
    nc.vector.reciprocal(oT[:, :, 0, :], e_v)

    gxy_r = sb["gxy"][:, 2 * J0:2 * (J0 + nch)] \
        .rearrange("p (c q) -> p c q", q=2)
    for kk in range(2):
        g_v = gxy_r[:, :, kk:kk + 1].broadcast_to([128, nch, 3])
        nc.vector.scalar_tensor_tensor(oT[:, :, 1 + kk, :],
                                       bx[:, :, :, 1 + kk], k, g_v,
                                       op0=OP.mult, op1=OP.add)

    twh = p_dec.tile([128, nch * 6], F32, tag=f"twh{tag}")
    twh_v = twh[:].rearrange("p (c q a) -> p c q a", c=nch, q=2)
    for kk in range(2):
        nc.scalar.activation(twh_v[:, :, kk, :], bx[:, :, :, 3 + kk], AF.Exp)
    anch_v = sb["anch"].rearrange("p (q a) -> p q a", q=2) \
        .unsqueeze(1).broadcast_to([128, nch, 2, 3])
    nc.vector.tensor_tensor(oT[:, :, 3:5, :], twh_v, anch_v, op=OP.mult)

    nc.scalar.activation(oT[:, :, 5, :], idxacc[:, J0:J0 + nch],
                         AF.Copy, bias=79.0, scale=-(2.0 ** 31))

    for a in range(3):
        cb = bx[:, :, a, 0:1].broadcast_to([128, nch, 6])
        dst = oA[:, :, a, :]
        nc.vector.scalar_tensor_tensor(dst, cb, 0.0, dst,
                                       op0=OP.is_gt, op1=OP.mult)

    if os.environ.get("KOUT", "scale") == "scale":
        nc.sync.dma_start(oX[:, 18 * J0:18 * (J0 + nch)],
                          outacc[:, 18 * J0:18 * (J0 + nch)])


def build():
    nc = bacc.Bacc("TRN2", target_bir_lowering=False, debug=False,
                   num_devices=N_CORES)
    xin = nc.dram_tensor("xin", [TOTAL_IN], F32, kind="ExternalInput").ap()
    oX = nc.dram_tensor("out", [128, OUT_W], F32,
                        kind="ExternalOutput").ap()

    with tile.TileContext(nc) as tc:
        with ExitStack() as ctx:
            p_c = ctx.enter_context(tc.tile_pool(name="consts", bufs=1))
            p_in = ctx.enter_context(tc.tile_pool(name="inp", bufs=2))
            p_cls = ctx.enter_context(tc.tile_pool(name="cls", bufs=2))
            p_m = ctx.enter_context(tc.tile_pool(name="small", bufs=2))
            p_dec = ctx.enter_context(tc.tile_pool(name="dec", bufs=1))
            p_acc = ctx.enter_context(tc.tile_pool(name="acc", bufs=1))

            sb = {}
            for name, shp in _CONST_SHAPES.items():
                t_ = p_c.tile(shp, F32, tag=name)
                size = shp[0] * shp[1]
                src = xin[_CONST_OFF[name]:_CONST_OFF[name] + size] \
                    .rearrange("(p f) -> p f", p=shp[0])
                nc.sync.dma_start(t_[:], src)
                sb[name] = t_[:]
            anch_t = sb["anch"]

            _st = int(os.environ.get("KSTAGE", "6"))
            boxacc = p_acc.tile([128, NCH_TOT * 15], F32, tag="boxacc")
            boxv = boxacc[:].rearrange("p (c a s) -> p c a s", a=3, s=5)
            idxacc = p_acc.tile([128, NCH_TOT * 3], F32, tag="idxacc")
            idxv = idxacc[:].rearrange("p (c a) -> p c a", a=3)
            outacc = p_acc.tile([128, OUT_W], F32, tag="outacc")
            if _st < 6:
                nc.vector.memset(outacc[:, :], 0.0)
                nc.vector.memset(boxacc[:, :], 0.0)
                nc.vector.memset(idxacc[:, :], 0.0)
            acc = {"pools": (p_in, p_cls, p_m, p_dec),
                   "boxacc": boxv, "idxacc": idxv, "outacc": outacc,
                   "oX": oX}

            for _rep in range(int(os.environ.get("KREP", "1"))):
                anch_off = 0
                for tag, h, t in SCALES:
                    sbs = dict(sb)
                    sbs["anch"] = anch_t[:, anch_off:anch_off + 6]
                    _emit_scale(nc, tc, sbs, acc, xin, h, t, tag)
                    anch_off += 6
                if _st < 6 or os.environ.get("KOUT", "scale") == "end":
                    nc.sync.dma_start(oX, outacc[:])
    nc.compile()
    return nc


_NC = None


def _get_nc():
    global _NC
    if _NC is None:
        _NC = build()
    return _NC


def _make_anch(anchors):
    anch = np.zeros((128, 18), np.float32)
    off = 0
    for tag, h, _ in SCALES:
        a = anchors[tag].astype(np.float64) / CASE
        for kk in range(2):
            for aa in range(3):
                anch[:, off + kk * 3 + aa] = a[aa, kk]
        off += 6
    return anch


def _pack_core(xs, anch):
    parts = []
    for tag, h, _ in SCALES:
        n = _cells(h)
        nch = NCH[tag]
        a = np.asarray(xs[tag]).reshape(B_PER, 255, h * h) \
            .transpose(0, 2, 1).reshape(n, 255)
        if nch * CHUNK > n:
            a = np.concatenate(
                [a, np.zeros((nch * CHUNK - n, 255), np.float32)], axis=0)
        a = np.ascontiguousarray(
            a.reshape(nch, CHUNK, 255).transpose(1, 0, 2))
        parts.append(a.ravel())
    parts += [_CONSTS["gxy"].ravel(), _CONSTS["iota"].ravel(), anch.ravel()]
    out = np.concatenate(parts)
    assert out.size == TOTAL_IN and out.dtype == np.float32
    return out


def _unpack_core(res):
    """[128, 4014] device tensor -> per-scale [n, 18] cell-major rows."""
    parts = []
    for tag, h, _ in SCALES[::-1]:  # output order: 13, 26, 52
        J0 = _J_OFF[tag]
        nch = NCH[tag]
        blk = res[:, 18 * J0:18 * (J0 + nch)].reshape(128, nch, 18)
        rows = blk.transpose(1, 0, 2).reshape(nch * CHUNK, 18)
        parts.append(rows[:_cells(h)])
    return parts


def kernel(out13, out26, out52, anchors13, anchors26, anchors52):
    nc = _get_nc()
    xs_all = {"13": np.asarray(out13), "26": np.asarray(out26),
              "52": np.asarray(out52)}
    anchors = {"13": np.asarray(anchors13), "26": np.asarray(anchors26),
               "52": np.asarray(anchors52)}
    anch = _make_anch(anchors)

    in_maps = []
    for i in range(N_CORES):
        xs = {tag: xs_all[tag][i * B_PER:(i + 1) * B_PER]
              for tag, _, _ in SCALES}
        in_maps.append({"xin": _pack_core(xs, anch)})

    res = run_bass_kernel_spmd(nc, in_maps, list(range(N_CORES))).results

    per_core = [_unpack_core(res[i]["out"]) for i in range(N_CORES)]
    parts = []
    for si in range(len(SCALES)):
        for i in range(N_CORES):
            parts.append(per_core[i][si].reshape(-1, 6))
    return np.concatenate(parts, axis=0)


# revision 11
# speedup vs baseline: 1.2566x; 1.2566x over previous
"""YOLO-style detection decode (nms_detection) on 8 trn2 NeuronCores.

Data-parallel over batch (64 -> 8 images/core). The host packs each
core's inputs into ONE flat f32 DRAM tensor, pre-transposed per scale
to cell-major chunks: [128 partitions, nch * 255] where partition p,
column 255*j + c holds channel c of cell j*128 + p. Every device load
is then one large contiguous-stride DMA, and no on-device transpose is
needed at all (the memory-regime roofline is the 29 MB input stream).
The result is ONE [128, 4014] f32 tensor (chunk-major cells on
partitions, 18 = 3 anchors x 6 box floats per cell), reassembled on
the host.

Device pipeline per 32-chunk group (chunk = 128 cells), all reads
straight from the input strip in SBUF:
  - scalar engine copies the 5 box channels (conf,x,y,w,h per anchor)
    into a per-scale accumulator.
  - DVE reduce_max over the 80 class cols per anchor -> m (exact).
  - DVE computes cls - m into an SBUF scratch (exact at the top:
    x - x = 0, Sterbenz near the max), then adds (79-c)*2^-31. The
    winner's value is exactly (79-argmax)*2^-31 >= 0 while every loser
    stays < 0, so a second DVE reduce_max recovers argmax exactly
    (incl. first-index ties, matching jnp.argmax).
  - Decode (sigmoid, grid offsets, exp*anchor, conf mask) runs ONCE
    per scale as ~10 wide DVE/ACT ops over the accumulators, writing
    the output accumulator in final layout; one DMA stores it.
"""

import os
from contextlib import ExitStack

import numpy as np

import concourse.bass as bass
import concourse.tile as tile
from concourse import bacc, mybir
from concourse.bass_utils import run_bass_kernel_spmd

N_CORES = 8
B = 64
B_PER = B // N_CORES
CASE = 416.0
SCALES = [("52", 52, 8.0), ("26", 26, 16.0), ("13", 13, 32.0)]
CHUNK = 128
LDC = 32           # chunks per SBUF load strip / compute group
F32 = mybir.dt.float32
AX = mybir.AxisListType
OP = mybir.AluOpType
AF = mybir.ActivationFunctionType
IOTA_SCALE = 2.0 ** -31


def _cells(h):
    return B_PER * h * h


def _nchunks(h):
    return (_cells(h) + CHUNK - 1) // CHUNK


NCH = {tag: _nchunks(h) for tag, h, _ in SCALES}
NCH_TOT = sum(NCH.values())          # 223
OUT_W = NCH_TOT * 18                 # 4014 f32 per partition


def _gxy_section(h, t):
    """[128, 2*nch] grid offsets: cols (2j, 2j+1) = (gx, gy) of chunk j."""
    n = _cells(h)
    nch = _nchunks(h)
    cells = np.arange(nch * CHUNK)
    s = cells % (h * h)
    gx = (s % h).astype(np.float64) * t / CASE
    gy = (s // h).astype(np.float64) * t / CASE
    gx[cells >= n] = 0.0
    gy[cells >= n] = 0.0
    out = np.zeros((CHUNK, 2 * nch), np.float32)
    for j in range(nch):
        out[:, 2 * j] = gx[j * CHUNK:(j + 1) * CHUNK]
        out[:, 2 * j + 1] = gy[j * CHUNK:(j + 1) * CHUNK]
    return out


def _consts():
    iota = np.broadcast_to(
        ((79.0 - np.arange(80)) * IOTA_SCALE).astype(np.float32), (128, 80))
    gxy = np.concatenate([_gxy_section(h, t) for _, h, t in SCALES], axis=1)
    return {"gxy": gxy.astype(np.float32),
            "iota": np.ascontiguousarray(iota)}


_CONSTS = _consts()
_CONST_SHAPES = {"gxy": [128, 2 * NCH_TOT], "iota": [128, 80],
                 "anch": [128, 18]}

# packed input layout (f32 elements, per core): per-scale cell-major
# chunked activations, then the small constants.
_X_OFF = {}
_off = 0
for _tag, _h, _t in SCALES:
    _X_OFF[_tag] = _off
    _off += NCH[_tag] * 255 * CHUNK
_CONST_OFF = {}
for _name in ("gxy", "iota", "anch"):
    _CONST_OFF[_name] = _off
    _off += int(np.prod(_CONST_SHAPES[_name]))
TOTAL_IN = _off

# chunk-column base per scale in the accumulators / output
_J_OFF = {}
_off = 0
for _tag, _h, _t in SCALES:
    _J_OFF[_tag] = _off
    _off += NCH[_tag]


def _emit_scale(nc, tc, sb, acc, xin, h, t, tag):
    ST = int(os.environ.get("KSTAGE", "6"))
    TT = getattr(nc, os.environ.get("KTTENG", "vector"))
    nch = NCH[tag]
    J0 = _J_OFF[tag]
    k = float(t / CASE)
    p_in, p_cls, p_m, p_dec = acc["pools"]
    boxacc, idxacc, outacc = acc["boxacc"], acc["idxacc"], acc["outacc"]

    # [128, nch, 255]: partition p, chunk j, channel c = cell j*128+p
    xc = xin[_X_OFF[tag]:_X_OFF[tag] + nch * 255 * CHUNK] \
        .rearrange("(p j c) -> p j c", p=128, c=255)

    for jb in range(0, nch, LDC):
        lc = min(LDC, nch - jb)

        in_a = p_in.tile([128, LDC * 255], F32, tag="in_a")
        ia = in_a[:].rearrange("p (j c) -> p j c", c=255)[:, 0:lc, :]
        nc.sync.dma_start(ia, xc[:, jb:jb + lc, :])
        if ST < 1:
            continue

        iv = ia.rearrange("p j (a r) -> p j a r", a=3)   # [128, lc, 3, 85]
        cls_in = iv[:, :, :, 5:85]
        J = J0 + jb

        # box channels (conf,x,y,w,h per anchor) -> accumulator
        m_sb = p_m.tile([128, LDC * 3], F32, tag="m_sb")
        m_v = m_sb[:].rearrange("p (j a) -> p j a", j=LDC)[:, 0:lc, :]
        if ST >= 2:
            nc.scalar.copy(boxacc[:, J:J + lc], iv[:, :, :, 0:5])
            # ---- scan 1: exact per-anchor class max ----
            nc.vector.tensor_reduce(m_v, cls_in, axis=AX.X, op=OP.max)
        else:
            nc.vector.memset(m_sb[:, :], 0.0)

        # ---- recenter into scratch: cls - m, then + iota payload ----
        cls_s = p_cls.tile([128, LDC * 240], F32, tag="cls_s")
        cv = cls_s[:].rearrange("p (j a r) -> p j a r", j=LDC, a=3)[:, 0:lc]
        if ST >= 3:
            m_b = m_v.unsqueeze(3).broadcast_to([128, lc, 3, 80])
            TT.scalar_tensor_tensor(cv, cls_in, 1.0, m_b,
                                    op0=OP.mult, op1=OP.subtract)
        else:
            nc.vector.memset(cls_s[:, :], 0.0)
        if ST >= 4:
            i_b = sb["iota"].unsqueeze(1).unsqueeze(1) \
                .broadcast_to([128, lc, 3, 80])
            TT.tensor_tensor(cv, cv, i_b, op=OP.add)

        # ---- scan 2: argmax payload ----
        if ST >= 5:
            nc.vector.tensor_reduce(idxacc[:, J:J + lc], cv,
                                    axis=AX.X, op=OP.max)

    # ---- batched decode over the whole scale ----
    if ST < 6:
        return
    oX = acc["oX"]
    oA = outacc[:].rearrange("p (c a s) -> p c a s", a=3, s=6)[:, J0:J0 + nch]
    oT = outacc[:].rearrange("p (c a s) -> p c s a", a=3, s=6)[:, J0:J0 + nch]
    bx = boxacc[:, J0:J0 + nch]                   # [128, nch, 3, 5]

    econf = p_dec.tile([128, nch * 3], F32, tag=f"econf{tag}")
    e_v = econf[:].rearrange("p (c a) -> p c a", c=nch)
    nc.scalar.activation(e_v, bx[:, :, :, 0], AF.Exp, scale=-1.0)
    nc.vector.tensor_scalar(e_v, e_v, 1.0, None, op0=OP.add)
    nc.vector.reciprocal(oT[:, :, 0, :], e_v)

    gxy_r = sb["gxy"][:, 2 * J0:2 * (J0 + nch)] \
        .rearrange("p (c q) -> p c q", q=2)
    for kk in range(2):
        g_v = gxy_r[:, :, kk:kk + 1].broadcast_to([128, nch, 3])
        nc.vector.scalar_tensor_tensor(oT[:, :, 1 + kk, :],
                                       bx[:, :, :, 1 + kk], k, g_v,
                                       op0=OP.mult, op1=OP.add)

    twh = p_dec.tile([128, nch * 6], F32, tag=f"twh{tag}")
    twh_v = twh[:].rearrange("p (c q a) -> p c q a", c=nch, q=2)
    for kk in range(2):
        nc.scalar.activation(twh_v[:, :, kk, :], bx[:, :, :, 3 + kk], AF.Exp)
    anch_v = sb["anch"].rearrange("p (q a) -> p q a", q=2) \
        .unsqueeze(1).broadcast_to([128, nch, 2, 3])
    nc.vector.tensor_tensor(oT[:, :, 3:5, :], twh_v, anch_v, op=OP.mult)

    nc.scalar.activation(oT[:, :, 5, :], idxacc[:, J0:J0 + nch],
                         AF.Copy, bias=79.0, scale=-(2.0 ** 31))

    for a in range(3):
        cb = bx[:, :, a, 0:1].broadcast_to([128, nch, 6])
        dst = oA[:, :, a, :]
        nc.vector.scalar_tensor_tensor(dst, cb, 0.0, dst,
                                       op0=OP.is_gt, op1=OP.mult)

    if os.environ.get("KOUT", "scale") == "scale":
        nc.sync.dma_start(oX[:, 18 * J0:18 * (J0 + nch)],
                          outacc[:, 18 * J0:18 * (J0 + nch)])


def build():
    nc = bacc.Bacc("TRN2", target_bir_lowering=False, debug=False,
                   num_devices=N_CORES)
    xin = nc.dram_tensor("xin", [TOTAL_IN], F32, kind="ExternalInput").ap()
    oX = nc.dram_tensor("out", [128, OUT_W], F32,
                        kind="ExternalOutput").ap()

    with tile.TileContext(nc) as tc:
        with ExitStack() as ctx:
            p_c = ctx.enter_context(tc.tile_pool(name="consts", bufs=1))
            p_in = ctx.enter_context(tc.tile_pool(
                name="inp", bufs=int(os.environ.get("KINBUFS", "2"))))
            p_cls = ctx.enter_context(tc.tile_pool(
                name="cls", bufs=int(os.environ.get("KCLSBUFS", "2"))))
            p_m = ctx.enter_context(tc.tile_pool(name="small", bufs=2))
            p_dec = ctx.enter_context(tc.tile_pool(name="dec", bufs=1))
            p_acc = ctx.enter_context(tc.tile_pool(name="acc", bufs=1))

            sb = {}
            for name, shp in _CONST_SHAPES.items():
                t_ = p_c.tile(shp, F32, tag=name)
                size = shp[0] * shp[1]
                src = xin[_CONST_OFF[name]:_CONST_OFF[name] + size] \
                    .rearrange("(p f) -> p f", p=shp[0])
                nc.sync.dma_start(t_[:], src)
                sb[name] = t_[:]
            anch_t = sb["anch"]

            _st = int(os.environ.get("KSTAGE", "6"))
            boxacc = p_acc.tile([128, NCH_TOT * 15], F32, tag="boxacc")
            boxv = boxacc[:].rearrange("p (c a s) -> p c a s", a=3, s=5)
            idxacc = p_acc.tile([128, NCH_TOT * 3], F32, tag="idxacc")
            idxv = idxacc[:].rearrange("p (c a) -> p c a", a=3)
            outacc = p_acc.tile([128, OUT_W], F32, tag="outacc")
            if _st < 6:
                nc.vector.memset(outacc[:, :], 0.0)
                nc.vector.memset(boxacc[:, :], 0.0)
                nc.vector.memset(idxacc[:, :], 0.0)
            acc = {"pools": (p_in, p_cls, p_m, p_dec),
                   "boxacc": boxv, "idxacc": idxv, "outacc": outacc,
                   "oX": oX}

            for _rep in range(int(os.environ.get("KREP", "1"))):
                anch_off = 0
                for tag, h, t in SCALES:
                    sbs = dict(sb)
                    sbs["anch"] = anch_t[:, anch_off:anch_off + 6]
                    _emit_scale(nc, tc, sbs, acc, xin, h, t, tag)
                    anch_off += 6
                if _st < 6 or os.environ.get("KOUT", "scale") == "end":
                    nc.sync.dma_start(oX, outacc[:])
    nc.compile()
    return nc


_NC = None


def _get_nc():
    global _NC
    if _NC is None:
        _NC = build()
    return _NC


def _make_anch(anchors):
    anch = np.zeros((128, 18), np.float32)
    off = 0
    for tag, h, _ in SCALES:
        a = anchors[tag].astype(np.float64) / CASE
        for kk in range(2):
            for aa in range(3):
                anch[:, off + kk * 3 + aa] = a[aa, kk]
        off += 6
    return anch


def _pack_core(xs, anch):
    parts = []
    for tag, h, _ in SCALES:
        n = _cells(h)
        nch = NCH[tag]
        a = np.asarray(xs[tag]).reshape(B_PER, 255, h * h) \
            .transpose(0, 2, 1).reshape(n, 255)
        if nch * CHUNK > n:
            a = np.concatenate(
                [a, np.zeros((nch * CHUNK - n, 255), np.float32)], axis=0)
        a = np.ascontiguousarray(
            a.reshape(nch, CHUNK, 255).transpose(1, 0, 2))
        parts.append(a.ravel())
    parts += [_CONSTS["gxy"].ravel(), _CONSTS["iota"].ravel(), anch.ravel()]
    out = np.concatenate(parts)
    assert out.size == TOTAL_IN and out.dtype == np.float32
    return out


def _unpack_core(res):
    """[128, 4014] device tensor -> per-scale [n, 18] cell-major rows."""
    parts = []
    for tag, h, _ in SCALES[::-1]:  # output order: 13, 26, 52
        J0 = _J_OFF[tag]
        nch = NCH[tag]
        blk = res[:, 18 * J0:18 * (J0 + nch)].reshape(128, nch, 18)
        rows = blk.transpose(1, 0, 2).reshape(nch * CHUNK, 18)
        parts.append(rows[:_cells(h)])
    return parts


def kernel(out13, out26, out52, anchors13, anchors26, anchors52):
    nc = _get_nc()
    xs_all = {"13": np.asarray(out13), "26": np.asarray(out26),
              "52": np.asarray(out52)}
    anchors = {"13": np.asarray(anchors13), "26": np.asarray(anchors26),
               "52": np.asarray(anchors52)}
    anch = _make_anch(anchors)

    in_maps = []
    for i in range(N_CORES):
        xs = {tag: xs_all[tag][i * B_PER:(i + 1) * B_PER]
              for tag, _, _ in SCALES}
        in_maps.append({"xin": _pack_core(xs, anch)})

    res = run_bass_kernel_spmd(nc, in_maps, list(range(N_CORES))).results

    per_core = [_unpack_core(res[i]["out"]) for i in range(N_CORES)]
    parts = []
    for si in range(len(SCALES)):
        for i in range(N_CORES):
            parts.append(per_core[i][si].reshape(-1, 6))
    return np.concatenate(parts, axis=0)
